# revision 3
# baseline (speedup 1.0000x reference)
"""Multi-head attention (B=8, S=1024, D=1024, H=16, dk=dv=64) on 8 TRN2 cores.

Sharding: data-parallel over batch — core b computes batch element b end to
end; no collectives. Host-side prep transposes activations/weights into the
layouts TensorE needs (contraction dim on partitions); all matmuls run on
device.

Per-core dataflow (everything "T" = [feature, seq] layout):
  qT[i,s] = sum_d WQT[d,i] * XQT[d,s]        (fp32r matmuls, N=512)
  kT      likewise; v[s,c] natural layout (XVT stationary)
  per head h:
    scoresT[s2,s1] = sum_j kT_h[j,s2] * qT_h[j,s1]   (K=64)
    expT = exp(scoresT/8)  on ScalarE (scale imm), bf16
    PV:  lhsT = [v_h | ones] (65 cols)  ->  psum[0:64,:]=ctx_unnorm^T,
         psum[64,:]= softmax denominator (free via the ones column)
    ctxT[c,s1] = psum[jv,s1] * recip(denominator)[s1]
  out[s1,m] = sum_c ctxT[c,s1] * WfcT[c,m]   (fp32r)
"""

import numpy as np

import concourse.bacc as bacc
import concourse.mybir as mybir
import concourse.tile as tile
from concourse.bass_utils import run_bass_kernel_spmd

S = 1024
D = 1024
H = 16
DK = 64
P = 128
NT = S // P          # 8 seq/feature tiles
NCH = 2              # 512-wide free-dim chunks
CH = S // NCH        # 512
F32 = mybir.dt.float32
F32R = mybir.dt.float32r
BF16 = mybir.dt.bfloat16
EXP = mybir.ActivationFunctionType.Exp

_CACHE = {}


def _build():
    nc = bacc.Bacc("TRN2", target_bir_lowering=False, debug=False)
    xqt = nc.dram_tensor("xqt", [D, S], F32R, kind="ExternalInput").ap()
    xkt = nc.dram_tensor("xkt", [D, S], F32R, kind="ExternalInput").ap()
    xvt = nc.dram_tensor("xvt", [D, S], F32R, kind="ExternalInput").ap()
    wqt = nc.dram_tensor("wqt", [D, D], F32R, kind="ExternalInput").ap()
    wkt = nc.dram_tensor("wkt", [D, D], F32R, kind="ExternalInput").ap()
    wvt = nc.dram_tensor("wvt", [D, D], F32R, kind="ExternalInput").ap()
    wft = nc.dram_tensor("wft", [D, D], F32R, kind="ExternalInput").ap()
    out = nc.dram_tensor("out", [S, D], F32, kind="ExternalOutput").ap()

    with tile.TileContext(nc) as tc:
        with (
            tc.tile_pool(name="persist", bufs=1) as pp,
            tc.tile_pool(name="psum", bufs=4, space="PSUM") as psp,
        ):
            qT = [pp.tile([P, S], F32R, tag=f"qT{t}", name=f"qT{t}") for t in range(NT)]
            kT = [pp.tile([P, S], F32R, tag=f"kT{t}", name=f"kT{t}") for t in range(NT)]
            # v in natural layout, bf16, with a ones column after each head:
            # col h*65+jv = v_h[:, jv], col h*65+64 = 1.0
            vpv = [pp.tile([P, H * (DK + 1)], BF16, tag=f"v{t}", name=f"v{t}") for t in range(NT)]
            ctxT = [pp.tile([P, S], F32R, tag=f"c{t}", name=f"c{t}") for t in range(NT)]

            with tc.tile_pool(name="ld", bufs=8) as lp:
                def load_tiles(src, tag, width):
                    ts = [lp.tile([P, width], F32R, tag=tag, name=tag) for _ in range(NT)]
                    for t in range(NT):
                        nc.sync.dma_start(out=ts[t][:], in_=src[t * P:(t + 1) * P, :])
                    return ts

                # ---- q/k projections: out[i, s] ----
                for src, wsrc, dst in ((xqt, wqt, qT), (xkt, wkt, kT)):
                    xts = load_tiles(src, "xt", S)
                    ws = load_tiles(wsrc, "w", D)
                    for i in range(NT):
                        pss = [psp.tile([P, CH], F32, tag="proj", name="proj") for _ in range(NCH)]
                        for d in range(NT):
                            for c in range(NCH):
                                nc.tensor.matmul(
                                    pss[c][:],
                                    lhsT=ws[d][:, i * P:(i + 1) * P],
                                    rhs=xts[d][:, c * CH:(c + 1) * CH],
                                    start=(d == 0),
                                    stop=(d == NT - 1),
                                )
                        for c in range(NCH):
                            nc.vector.tensor_copy(
                                dst[i][:, c * CH:(c + 1) * CH], pss[c][:]
                            )

                # ---- v projection: natural out[s2, c] (XVT stationary) ----
                xts = load_tiles(xvt, "xt", S)
                ws = load_tiles(wvt, "w", D)
                for s2 in range(NT):
                    pss = [psp.tile([P, CH], F32, tag="proj", name="proj") for _ in range(NCH)]
                    for d in range(NT):
                        for c in range(NCH):
                            nc.tensor.matmul(
                                pss[c][:],
                                lhsT=xts[d][:, s2 * P:(s2 + 1) * P],
                                rhs=ws[d][:, c * CH:(c + 1) * CH],
                                start=(d == 0),
                                stop=(d == NT - 1),
                            )
                    # ones columns (head stride 65, col 64)
                    nc.vector.memset(
                        vpv[s2][:, 0:H * 65].rearrange(
                            "p (h x) -> p h x", x=65)[:, :, 64:65],
                        1.0,
                    )
                    for c in range(NCH):
                        # psum [128, 512] = heads 8c..8c+7 -> strided bf16 store
                        dst_ap = vpv[s2][:, c * 520:(c + 1) * 520].rearrange(
                            "p (h x) -> p h x", x=65)[:, :, 0:64]
                        src_ap = pss[c][:].rearrange("p (h x) -> p h x", x=64)
                        nc.vector.tensor_copy(dst_ap, src_ap)

            with tc.tile_pool(name="fc", bufs=8) as fp:
                wf = [fp.tile([P, D], F32R, tag="wf", name="wf") for _ in range(NT)]
                for t in range(NT):
                    nc.sync.dma_start(out=wf[t][:], in_=wft[t * P:(t + 1) * P, :])

                with tc.tile_pool(name="attn", bufs=2) as ap_:
                    for h in range(H):
                        t2, off = h // 2, (h % 2) * DK
                        expT = [ap_.tile([P, S], BF16, tag=f"exp{t}", name=f"exp{t}")
                                for t in range(NT)]
                        # scoresT[s2, s1] then exp(x/8) -> bf16
                        for s2 in range(NT):
                            for c in range(NCH):
                                ps = psp.tile([P, CH], F32, tag="sc", name="sc", bufs=2)
                                nc.tensor.matmul(
                                    ps[:],
                                    lhsT=kT[t2][off:off + DK,
                                                s2 * P:(s2 + 1) * P],
                                    rhs=qT[t2][off:off + DK,
                                               c * CH:(c + 1) * CH],
                                    start=True,
                                    stop=True,
                                )
                                nc.scalar.activation(
                                    expT[s2][:, c * CH:(c + 1) * CH], ps[:],
                                    EXP, scale=0.125,
                                )
                        # PV: ctx_unnorm^T in psum[0:64], denominator in row 64
                        for c in range(NCH):
                            ps = psp.tile([P, CH], F32, tag="pv", name="pv", bufs=2)
                            for s2 in range(NT):
                                nc.tensor.matmul(
                                    ps[0:DK + 1, :],
                                    lhsT=vpv[s2][:, h * 65:(h + 1) * 65],
                                    rhs=expT[s2][:, c * CH:(c + 1) * CH],
                                    start=(s2 == 0),
                                    stop=(s2 == NT - 1),
                                )
                            rr = ap_.tile([1, CH], F32, tag="rr", name="rr")
                            nc.vector.reciprocal(rr[:], ps[DK:DK + 1, :])
                            rb = ap_.tile([DK, CH], F32, tag="rb", name="rb")
                            nc.gpsimd.partition_broadcast(rb[:], rr[:])
                            nc.vector.tensor_mul(
                                ctxT[t2][off:off + DK, c * CH:(c + 1) * CH],
                                ps[0:DK, :], rb[:],
                            )

                # ---- fc: out[s1, m] ----
                for s1 in range(NT):
                    pss = [psp.tile([P, CH], F32, tag="proj", name="proj") for _ in range(NCH)]
                    for ct in range(NT):
                        for c in range(NCH):
                            nc.tensor.matmul(
                                pss[c][:],
                                lhsT=ctxT[ct][:, s1 * P:(s1 + 1) * P],
                                rhs=wf[ct][:, c * CH:(c + 1) * CH],
                                start=(ct == 0),
                                stop=(ct == NT - 1),
                            )
                    for c in range(NCH):
                        ob = fp.tile([P, CH], F32, tag="ob", name="ob", bufs=4)
                        nc.vector.tensor_copy(ob[:], pss[c][:])
                        nc.sync.dma_start(
                            out=out[s1 * P:(s1 + 1) * P, c * CH:(c + 1) * CH],
                            in_=ob[:],
                        )

    nc.compile()
    return nc


def run(inputs, trace=False):
    """inputs: dict with Q,K,V [8,1024,1024] and WQ,WK,WV,Wfc [1024,1024].
    Returns (out [8,1024,1024] fp32, exec_time_ns or None)."""
    if "nc" not in _CACHE:
        _CACHE["nc"] = _build()
    nc = _CACHE["nc"]

    f32 = np.float32
    wqt = np.ascontiguousarray(np.asarray(inputs["WQ"], dtype=f32).T)
    wkt = np.ascontiguousarray(np.asarray(inputs["WK"], dtype=f32).T)
    wvt = np.ascontiguousarray(np.asarray(inputs["WV"], dtype=f32).T)
    wft = np.ascontiguousarray(np.asarray(inputs["Wfc"], dtype=f32).T)
    Q = np.asarray(inputs["Q"], dtype=f32)
    K = np.asarray(inputs["K"], dtype=f32)
    V = np.asarray(inputs["V"], dtype=f32)

    in_maps = [
        {
            "xqt": np.ascontiguousarray(Q[b].T),
            "xkt": np.ascontiguousarray(K[b].T),
            "xvt": np.ascontiguousarray(V[b].T),
            "wqt": wqt, "wkt": wkt, "wvt": wvt, "wft": wft,
        }
        for b in range(8)
    ]
    res = run_bass_kernel_spmd(nc, in_maps, core_ids=list(range(8)), trace=trace)
    out = np.stack([res.results[b]["out"] for b in range(8)], axis=0)
    return out.astype(np.float32), res.exec_time_ns


def kernel(**inputs):
    return run(inputs, trace=False)[0]


# revision 5
# speedup vs baseline: 1.1404x; 1.1404x over previous
"""Multi-head attention (B=8, S=1024, D=1024, H=16, dk=dv=64) on 8 TRN2 cores.

Sharding: data-parallel over batch — core b computes batch element b end to
end; no collectives. Host-side prep transposes activations/weights into the
layouts TensorE needs (contraction dim on partitions); all matmuls run on
device.

Per-core dataflow (everything "T" = [feature, seq] layout):
  qT[i,s] = sum_d WQT[d,i] * XQT[d,s]        (fp32r matmuls, N=512)
  kT      likewise; v[s,c] natural layout (XVT stationary)
  per head h:
    scoresT[s2,s1] = sum_j kT_h[j,s2] * qT_h[j,s1]   (K=64)
    expT = exp(scoresT/8)  on ScalarE (scale imm), bf16
    PV:  lhsT = [v_h | ones] (65 cols)  ->  psum[0:64,:]=ctx_unnorm^T,
         psum[64,:]= softmax denominator (free via the ones column)
    ctxT[c,s1] = psum[jv,s1] * recip(denominator)[s1]
  out[s1,m] = sum_c ctxT[c,s1] * WfcT[c,m]   (fp32r)
"""

import numpy as np

import concourse.bacc as bacc
import concourse.mybir as mybir
import concourse.tile as tile
from concourse.bass_utils import run_bass_kernel_spmd

S = 1024
D = 1024
H = 16
DK = 64
P = 128
NT = S // P          # 8 seq/feature tiles
NCH = 2              # 512-wide free-dim chunks
CH = S // NCH        # 512
F32 = mybir.dt.float32
F32R = mybir.dt.float32r
BF16 = mybir.dt.bfloat16
EXP = mybir.ActivationFunctionType.Exp

_CACHE = {}


def _build():
    nc = bacc.Bacc("TRN2", target_bir_lowering=False, debug=False)
    xqt = nc.dram_tensor("xqt", [D, S], BF16, kind="ExternalInput").ap()
    xkt = nc.dram_tensor("xkt", [D, S], BF16, kind="ExternalInput").ap()
    xvt = nc.dram_tensor("xvt", [D, S], BF16, kind="ExternalInput").ap()
    wqt = nc.dram_tensor("wqt", [D, D], BF16, kind="ExternalInput").ap()
    wkt = nc.dram_tensor("wkt", [D, D], BF16, kind="ExternalInput").ap()
    wvt = nc.dram_tensor("wvt", [D, D], BF16, kind="ExternalInput").ap()
    wft = nc.dram_tensor("wft", [D, D], BF16, kind="ExternalInput").ap()
    out = nc.dram_tensor("out", [S, D], F32, kind="ExternalOutput").ap()

    with tile.TileContext(nc) as tc:
        with (
            tc.tile_pool(name="persist", bufs=1) as pp,
            tc.tile_pool(name="psum", bufs=4, space="PSUM") as psp,
        ):
            qT = [pp.tile([P, S], BF16, tag=f"qT{t}", name=f"qT{t}") for t in range(NT)]
            kT = [pp.tile([P, S], BF16, tag=f"kT{t}", name=f"kT{t}") for t in range(NT)]
            # v in natural layout, bf16, with a ones column after each head:
            # col h*65+jv = v_h[:, jv], col h*65+64 = 1.0
            vpv = [pp.tile([P, H * (DK + 1)], BF16, tag=f"v{t}", name=f"v{t}") for t in range(NT)]
            ctxT = [pp.tile([P, S], BF16, tag=f"c{t}", name=f"c{t}") for t in range(NT)]

            with tc.tile_pool(name="ld", bufs=8) as lp:
                def load_tiles(src, tag, width):
                    ts = [lp.tile([P, width], BF16, tag=tag, name=tag) for _ in range(NT)]
                    for t in range(NT):
                        nc.sync.dma_start(out=ts[t][:], in_=src[t * P:(t + 1) * P, :])
                    return ts

                # ---- q/k projections: out[i, s] ----
                for src, wsrc, dst in ((xqt, wqt, qT), (xkt, wkt, kT)):
                    xts = load_tiles(src, "xt", S)
                    ws = load_tiles(wsrc, "w", D)
                    for i in range(NT):
                        pss = [psp.tile([P, CH], F32, tag="proj", name="proj") for _ in range(NCH)]
                        for d in range(NT):
                            for c in range(NCH):
                                nc.tensor.matmul(
                                    pss[c][:],
                                    lhsT=ws[d][:, i * P:(i + 1) * P],
                                    rhs=xts[d][:, c * CH:(c + 1) * CH],
                                    start=(d == 0),
                                    stop=(d == NT - 1),
                                )
                        for c in range(NCH):
                            nc.vector.tensor_copy(
                                dst[i][:, c * CH:(c + 1) * CH], pss[c][:]
                            )

                # ---- v projection: natural out[s2, c] (XVT stationary) ----
                xts = load_tiles(xvt, "xt", S)
                ws = load_tiles(wvt, "w", D)
                for s2 in range(NT):
                    pss = [psp.tile([P, CH], F32, tag="proj", name="proj") for _ in range(NCH)]
                    for d in range(NT):
                        for c in range(NCH):
                            nc.tensor.matmul(
                                pss[c][:],
                                lhsT=xts[d][:, s2 * P:(s2 + 1) * P],
                                rhs=ws[d][:, c * CH:(c + 1) * CH],
                                start=(d == 0),
                                stop=(d == NT - 1),
                            )
                    # ones columns (head stride 65, col 64)
                    nc.vector.memset(
                        vpv[s2][:, 0:H * 65].rearrange(
                            "p (h x) -> p h x", x=65)[:, :, 64:65],
                        1.0,
                    )
                    for c in range(NCH):
                        # psum [128, 512] = heads 8c..8c+7 -> strided bf16 store
                        dst_ap = vpv[s2][:, c * 520:(c + 1) * 520].rearrange(
                            "p (h x) -> p h x", x=65)[:, :, 0:64]
                        src_ap = pss[c][:].rearrange("p (h x) -> p h x", x=64)
                        nc.vector.tensor_copy(dst_ap, src_ap)

            with tc.tile_pool(name="fc", bufs=8) as fp:
                wf = [fp.tile([P, D], BF16, tag="wf", name="wf") for _ in range(NT)]
                for t in range(NT):
                    nc.sync.dma_start(out=wf[t][:], in_=wft[t * P:(t + 1) * P, :])

                with tc.tile_pool(name="attn", bufs=2) as ap_:
                    for h in range(H):
                        t2, off = h // 2, (h % 2) * DK
                        expT = [ap_.tile([P, S], BF16, tag=f"exp{t}", name=f"exp{t}")
                                for t in range(NT)]
                        # scoresT[s2, s1] then exp(x/8) -> bf16
                        for s2 in range(NT):
                            for c in range(NCH):
                                ps = psp.tile([P, CH], F32, tag="sc", name="sc", bufs=2)
                                nc.tensor.matmul(
                                    ps[:],
                                    lhsT=kT[t2][off:off + DK,
                                                s2 * P:(s2 + 1) * P],
                                    rhs=qT[t2][off:off + DK,
                                               c * CH:(c + 1) * CH],
                                    start=True,
                                    stop=True,
                                )
                                nc.scalar.activation(
                                    expT[s2][:, c * CH:(c + 1) * CH], ps[:],
                                    EXP, scale=0.125,
                                )
                        # PV: ctx_unnorm^T in psum[0:64], denominator in row 64
                        for c in range(NCH):
                            ps = psp.tile([P, CH], F32, tag="pv", name="pv", bufs=2)
                            for s2 in range(NT):
                                nc.tensor.matmul(
                                    ps[0:DK + 1, :],
                                    lhsT=vpv[s2][:, h * 65:(h + 1) * 65],
                                    rhs=expT[s2][:, c * CH:(c + 1) * CH],
                                    start=(s2 == 0),
                                    stop=(s2 == NT - 1),
                                )
                            rr = ap_.tile([1, CH], F32, tag="rr", name="rr")
                            nc.vector.reciprocal(rr[:], ps[DK:DK + 1, :])
                            rb = ap_.tile([DK, CH], F32, tag="rb", name="rb")
                            nc.gpsimd.partition_broadcast(rb[:], rr[:])
                            nc.vector.tensor_mul(
                                ctxT[t2][off:off + DK, c * CH:(c + 1) * CH],
                                ps[0:DK, :], rb[:],
                            )

                # ---- fc: out[s1, m] ----
                for s1 in range(NT):
                    pss = [psp.tile([P, CH], F32, tag="proj", name="proj") for _ in range(NCH)]
                    for ct in range(NT):
                        for c in range(NCH):
                            nc.tensor.matmul(
                                pss[c][:],
                                lhsT=ctxT[ct][:, s1 * P:(s1 + 1) * P],
                                rhs=wf[ct][:, c * CH:(c + 1) * CH],
                                start=(ct == 0),
                                stop=(ct == NT - 1),
                            )
                    for c in range(NCH):
                        ob = fp.tile([P, CH], F32, tag="ob", name="ob", bufs=4)
                        nc.vector.tensor_copy(ob[:], pss[c][:])
                        nc.sync.dma_start(
                            out=out[s1 * P:(s1 + 1) * P, c * CH:(c + 1) * CH],
                            in_=ob[:],
                        )

    nc.compile()
    return nc


def run(inputs, trace=False):
    """inputs: dict with Q,K,V [8,1024,1024] and WQ,WK,WV,Wfc [1024,1024].
    Returns (out [8,1024,1024] fp32, exec_time_ns or None)."""
    if "nc" not in _CACHE:
        _CACHE["nc"] = _build()
    nc = _CACHE["nc"]

    import ml_dtypes
    bf16 = ml_dtypes.bfloat16
    f32 = np.float32
    wqt = np.ascontiguousarray(np.asarray(inputs["WQ"], dtype=f32).T.astype(bf16))
    wkt = np.ascontiguousarray(np.asarray(inputs["WK"], dtype=f32).T.astype(bf16))
    wvt = np.ascontiguousarray(np.asarray(inputs["WV"], dtype=f32).T.astype(bf16))
    wft = np.ascontiguousarray(np.asarray(inputs["Wfc"], dtype=f32).T.astype(bf16))
    Q = np.asarray(inputs["Q"], dtype=f32)
    K = np.asarray(inputs["K"], dtype=f32)
    V = np.asarray(inputs["V"], dtype=f32)

    in_maps = [
        {
            "xqt": np.ascontiguousarray(Q[b].T.astype(bf16)),
            "xkt": np.ascontiguousarray(K[b].T.astype(bf16)),
            "xvt": np.ascontiguousarray(V[b].T.astype(bf16)),
            "wqt": wqt, "wkt": wkt, "wvt": wvt, "wft": wft,
        }
        for b in range(8)
    ]
    res = run_bass_kernel_spmd(nc, in_maps, core_ids=list(range(8)), trace=trace)
    out = np.stack([res.results[b]["out"] for b in range(8)], axis=0)
    return out.astype(np.float32), res.exec_time_ns


def kernel(**inputs):
    return run(inputs, trace=False)[0]
